# revision 11
# baseline (speedup 1.0000x reference)
"""DualRelGCN message-passing kernel for 8 TRN2 NeuronCores.

Strategy (destination-sharded, collective-free, X-stationary block-dense):
  - LayerNorm is invariant to positive per-row scaling, so LN(agg/denom) ==
    LN(agg): the denominator drops out entirely.
  - Shard edges by dst range: core c owns nodes [1250c, 1250(c+1)).  Each
    core computes its 1250 output rows locally -> no collectives.
  - agg^T[dim, dst] = sum_j X_j^T @ W_j: X pair-tiles (fp8, SBUF-resident)
    are the STATIONARY matmul operand, reused across all dst columns; the
    dense weighted-adjacency blocks W_j (fp8, host-scattered) stream as the
    MOVING operand.  This needs only 2 LDWEIGHTS per src pair (160 total)
    instead of one per matmul (400) -- redundant InstLdweights emitted by
    the compiler are deleted post-compile (stationary persists in the PE).
  - The [256, 1280] agg^T accumulates in PSUM across the whole src sweep
    (10 fp32 [128,256] slices = 5 banks).  Sweep is DMA-bound (13.1MB of
    W blocks) and PE idles ~40%: the weight stream is the roofline.
  - Epilogue per dst tile: evacuate agg^T, PE-transpose to row layout,
    LN (fused two-scalar ops, Rsqrt), y = lnT @ proj_w^T, out = rel + 0.1y.
    rel/out ride in bf16 (host casts) to halve their HBM traffic.
"""

import sys

for _p in ("/opt/trn_rl_repo",):
    if _p not in sys.path:
        sys.path.insert(0, _p)

from contextlib import ExitStack

import numpy as np
import ml_dtypes

import concourse.bacc as bacc
import concourse.mybir as mybir
from concourse.alu_op_type import AluOpType
from concourse.tile import TileContext
from concourse.bass_utils import run_bass_kernel_spmd

F32 = mybir.dt.float32
BF16 = mybir.dt.bfloat16
FP8 = mybir.dt.float8e4
AF = mybir.ActivationFunctionType
DR = mybir.MatmulPerfMode.DoubleRow

N_NODES = 10000
DIM = 256
N_CORES = 8
NODES_PER_CORE = N_NODES // N_CORES  # 1250
TILE = 128
N_TILES = 10            # dst tiles per core (1280 rows incl. 30 pad)
OUT_ROWS = N_TILES * TILE
S_TILES = 80            # 79 src tiles + 1 zero pad -> even count
S_PAIRS = S_TILES // 2  # 40 DoubleRow pairs
N_GROUPS = 5            # dst column groups of 256 (moving free dim limit)
ALPHA = 0.1
LN_EPS = 1e-5

_CACHE: dict = {}


def _dedup_ldweights(nc):
    """Delete InstLdweights whose AP matches the previously loaded one.

    The compiler emits one Ldweights per matmul; when consecutive matmuls
    share a stationary operand the PE array already holds it.  Only drops
    duplicates that carry no semaphore waits/updates.
    """
    removed = 0
    for fn in nc.m.functions:
        for blk in fn.blocks:
            insts = blk.instructions
            last_sig = None
            keep = []
            changed = False
            for i in insts:
                tn = type(i).__name__
                if tn == "InstLdweights":
                    si = i.sync_info
                    clean = si is None or (
                        len(si.on_wait) == 0 and len(si.on_update) == 0)
                    sig = (str(i.ins[0]), str(i.perf_mode),
                           str(i.is_transpose))
                    if clean and sig == last_sig:
                        removed += 1
                        changed = True
                        continue
                    last_sig = sig
                elif tn == "InstMatmult":
                    if getattr(i, "is_transpose", False):
                        last_sig = None
                elif tn in ("InstEventSemaphore", "InstDrain", "InstNop",
                            "InstNotify"):
                    pass  # does not clobber the PE weight array
                elif tn == "InstMatmultMx":
                    last_sig = None
                keep.append(i)
            if changed:
                while len(insts):
                    insts.pop()
                for i in keep:
                    insts.append(i)
    return removed


def _build():
    nc = bacc.Bacc("TRN2", target_bir_lowering=False, debug=False,
                   num_devices=N_CORES)

    x_d = nc.dram_tensor("x", [128, S_TILES * DIM], FP8,
                         kind="ExternalInput")
    w_d = nc.dram_tensor("wblk", [S_PAIRS, 128, 2 * OUT_ROWS], FP8,
                         kind="ExternalInput")
    rel_d = nc.dram_tensor("relslice", [OUT_ROWS, DIM], BF16,
                           kind="ExternalInput")
    pwt_d = nc.dram_tensor("projwT", [128, 2 * DIM], BF16,
                           kind="ExternalInput")
    out_d = nc.dram_tensor("out", [OUT_ROWS, DIM], BF16,
                           kind="ExternalOutput")

    with TileContext(nc) as tc, ExitStack() as es:
        const_pool = es.enter_context(tc.tile_pool(name="const", bufs=1))
        wpool = es.enter_context(tc.tile_pool(name="wblk", bufs=3))
        ep_pool = es.enter_context(tc.tile_pool(name="ep", bufs=2))
        ps_agg = es.enter_context(tc.tile_pool(name="ps_agg", bufs=1,
                                               space="PSUM"))
        ps_row = es.enter_context(tc.tile_pool(name="ps_row", bufs=1,
                                               space="PSUM"))
        ps_tr = es.enter_context(tc.tile_pool(name="ps_tr", bufs=1,
                                              space="PSUM"))
        ps_y = es.enter_context(tc.tile_pool(name="ps_y", bufs=1,
                                             space="PSUM"))

        # --- constants / resident inputs ---
        iota_row = const_pool.tile([128, 128], F32, tag="iota")
        nc.gpsimd.iota(iota_row[:], [[1, 128]], base=0, channel_multiplier=0,
                       allow_small_or_imprecise_dtypes=True)
        pidx = const_pool.tile([128, 1], F32, tag="pidx")
        nc.gpsimd.iota(pidx[:], [[1, 1]], base=0, channel_multiplier=1,
                       allow_small_or_imprecise_dtypes=True)
        ident = const_pool.tile([128, 128], BF16, tag="ident")
        nc.vector.tensor_scalar(ident[:], iota_row[:], pidx[:], None,
                                AluOpType.is_equal)
        epsb = const_pool.tile([128, 1], F32, tag="epsb")
        nc.vector.memset(epsb[:], LN_EPS)
        pwt_sb = const_pool.tile([128, 2, DIM], BF16, tag="pwt")
        nc.scalar.dma_start(pwt_sb[:], pwt_d[:])
        rel_sb = const_pool.tile([128, N_TILES, DIM], BF16, tag="rel")

        # X fp8, fully resident as the stationary operand; chunked load on
        # the scalar ring so the first pairs land before the sweep starts.
        x_sb = const_pool.tile([128, S_TILES, DIM], FP8, tag="x")
        XC = 10  # pairs of tiles per chunk... (20 tiles)
        for i in range(0, S_TILES, 2 * XC):
            nc.scalar.dma_start(x_sb[:, i:i + 2 * XC, :],
                                x_d[:, i * DIM:(i + 2 * XC) * DIM])

        # rel slices (bf16) early on the scalar ring, one per dst tile.
        for t in range(N_TILES):
            nc.scalar.dma_start(
                rel_sb[:, t, :],
                rel_d[t * 128:(t + 1) * 128, :])

        # --- src sweep: agg^T accumulates in PSUM --------------------------
        # slot (h, g) holds dims [128h,128h+128) x dst cols [256g, 256g+256)
        aggT = ps_agg.tile([128, 2 * N_GROUPS, 256], F32, tag="aggT")
        for j in range(S_PAIRS):
            w_t = wpool.tile([128, 2, OUT_ROWS], FP8, tag="w")
            nc.sync.dma_start(w_t[:], w_d[j])
            for h in range(2):
                lhs = x_sb[:, 2 * j:2 * j + 2, 128 * h:128 * (h + 1)]
                for g in range(N_GROUPS):
                    nc.tensor.matmul(
                        aggT[:, h * N_GROUPS + g, :],
                        lhs,
                        w_t[:, :, 256 * g:256 * (g + 1)],
                        start=(j == 0), stop=(j == S_PAIRS - 1),
                        perf_mode=DR)

        # --- epilogue ------------------------------------------------------
        for t in range(N_TILES):
            g, half = t // 2, t % 2
            # evacuate agg^T [2*128 dim, 128 dst] to SBUF bf16 (for PE
            # transpose input), split across ACT and Pool engines
            aggT_sb = ep_pool.tile([128, 2, 128], BF16, tag="aggT_sb")
            nc.scalar.copy(aggT_sb[:, 0, :],
                           aggT[:, 0 * N_GROUPS + g,
                                128 * half:128 * half + 128])
            nc.vector.tensor_copy(aggT_sb[:, 1, :],
                                  aggT[:, 1 * N_GROUPS + g,
                                       128 * half:128 * half + 128])
            # transpose to row layout [128 dst, 256 dim] (PSUM)
            agg_row = ps_row.tile([128, 2, 128], BF16, tag="agg_row")
            for h in range(2):
                nc.tensor.transpose(agg_row[:, h, :], aggT_sb[:, h, :],
                                    ident[:])
            # LN: rowsum fused into the PSUM->SBUF copy
            agg_sb = ep_pool.tile([128, DIM], BF16, tag="agg_sb")
            rowsum = ep_pool.tile([128, 1], F32, tag="rowsum")
            nc.scalar.activation(agg_sb[:], agg_row[:], AF.Copy,
                                 accum_out=rowsum[:])
            mean = ep_pool.tile([128, 1], F32, tag="mean")
            nc.scalar.mul(mean[:], rowsum[:], 1.0 / DIM)
            cent = ep_pool.tile([128, DIM], BF16, tag="cent")
            nc.vector.tensor_scalar(cent[:], agg_sb[:], mean[:], None,
                                    AluOpType.subtract)
            sq = ep_pool.tile([128, DIM], BF16, tag="sq")
            sumsq = ep_pool.tile([128, 1], F32, tag="sumsq")
            nc.scalar.activation(sq[:], cent[:], AF.Square,
                                 accum_out=sumsq[:])
            std = ep_pool.tile([128, 1], F32, tag="std")
            nc.scalar.activation(std[:], sumsq[:], AF.Sqrt, bias=epsb[:],
                                 scale=1.0 / DIM)
            rstd = ep_pool.tile([128, 1], F32, tag="rstd")
            nc.vector.reciprocal(rstd[:], std[:])
            ln = ep_pool.tile([128, DIM], BF16, tag="ln")
            nc.vector.tensor_scalar(ln[:], cent[:], rstd[:], None,
                                    AluOpType.mult)

            # y = ln @ proj_w.T via PE transpose + matmul
            y_ps = ps_y.tile([128, DIM], F32, tag="y")
            for k in range(2):
                tr_ps = ps_tr.tile([128, 128], BF16, tag="tr")
                nc.tensor.transpose(tr_ps[:], ln[:, k * 128:(k + 1) * 128],
                                    ident[:])
                lnT = ep_pool.tile([128, 128], BF16, tag="lnT")
                nc.scalar.copy(lnT[:], tr_ps[:])
                nc.tensor.matmul(y_ps[:], lnT[:], pwt_sb[:, k, :],
                                 start=(k == 0), stop=(k == 1))

            # out = rel + alpha*y in one fused DVE op
            out_t = ep_pool.tile([128, DIM], BF16, tag="out")
            nc.vector.scalar_tensor_tensor(
                out_t[:], y_ps[:], ALPHA, rel_sb[:, t, :],
                AluOpType.mult, AluOpType.add)
            nc.scalar.dma_start(out_d[t * 128:(t + 1) * 128, :], out_t[:])

    nc.compile()
    import os
    if os.environ.get("NO_DEDUP") != "1":
        _dedup_ldweights(nc)
    return nc


def _prep(rel_embed, rel_edge_index, rel_edge_weight, proj_w):
    """Host-side sharding/layout: scatter edges into dense per-(src pair)
    weight blocks; lay out rel_embed for SBUF residency."""
    src = np.asarray(rel_edge_index[0], dtype=np.int64)
    dst = np.asarray(rel_edge_index[1], dtype=np.int64)
    w = np.asarray(rel_edge_weight, dtype=np.float32)
    rel = np.asarray(rel_embed, dtype=np.float32)
    pw = np.asarray(proj_w, dtype=np.float32)

    core = dst // NODES_PER_CORE
    drel = dst - core * NODES_PER_CORE
    t = drel // TILE
    d = drel % TILE
    s = src // TILE
    p = src % TILE
    # flat index inside one core's [N_TILES, S_TILES, 128, 128] block array
    flat = ((t * S_TILES + s) * TILE + p) * TILE + d
    blk_sz = N_TILES * S_TILES * TILE * TILE

    w_dev = np.empty((N_CORES, S_PAIRS, 128, 2 * OUT_ROWS),
                     dtype=ml_dtypes.float8_e4m3)
    for c in range(N_CORES):
        m = core == c
        wc = np.bincount(flat[m], weights=w[m], minlength=blk_sz)
        wc = wc.reshape(N_TILES, S_TILES, TILE, TILE).astype(np.float32)
        # [t, s, p, d] -> [j, p, (q, t*128+d)]
        arr = wc.transpose(1, 2, 0, 3).reshape(S_TILES, 128, OUT_ROWS)
        arr = arr.reshape(S_PAIRS, 2, 128, OUT_ROWS).transpose(0, 2, 1, 3)
        w_dev[c] = arr.reshape(S_PAIRS, 128, 2 * OUT_ROWS)

    rel8 = rel.astype(ml_dtypes.float8_e4m3)
    rel8_pad = np.zeros((S_TILES * TILE, DIM), dtype=ml_dtypes.float8_e4m3)
    rel8_pad[:N_NODES] = rel8
    x_dev = np.ascontiguousarray(
        rel8_pad.reshape(S_TILES, TILE, DIM).transpose(1, 0, 2).reshape(
            128, S_TILES * DIM))

    relslice = np.zeros((N_CORES, OUT_ROWS, DIM), dtype=ml_dtypes.bfloat16)
    for c in range(N_CORES):
        relslice[c, :NODES_PER_CORE] = rel[c * NODES_PER_CORE:
                                           (c + 1) * NODES_PER_CORE]
    pwt = pw.T.astype(ml_dtypes.bfloat16)  # [f, o]
    pwt_dev = np.ascontiguousarray(
        pwt.reshape(2, 128, DIM).transpose(1, 0, 2).reshape(128, 2 * DIM))

    in_maps = []
    for c in range(N_CORES):
        in_maps.append({
            "x": x_dev,
            "wblk": w_dev[c],
            "relslice": relslice[c],
            "projwT": pwt_dev,
        })
    return in_maps


def kernel(rel_embed, rel_edge_index, rel_edge_weight, proj_w,
           _trace=False):
    in_maps = _prep(rel_embed, rel_edge_index, rel_edge_weight, proj_w)
    nc = _CACHE.get("nc")
    if nc is None:
        nc = _build()
        _CACHE["nc"] = nc
    res = run_bass_kernel_spmd(nc, in_maps, core_ids=list(range(N_CORES)),
                               trace=_trace)
    out = np.concatenate(
        [res.results[c]["out"][:NODES_PER_CORE] for c in range(N_CORES)],
        axis=0)
    if _trace:
        kernel.last_results = res
    return out.astype(np.float32)


# revision 18
# speedup vs baseline: 1.2211x; 1.2211x over previous
"""DualRelGCN message-passing kernel for 8 TRN2 NeuronCores.

Strategy (destination-sharded, collective-free, X-stationary block-dense):
  - LayerNorm is invariant to positive per-row scaling, so LN(agg/denom) ==
    LN(agg): the denominator drops out entirely.
  - Shard edges by dst range: core c owns nodes [1250c, 1250(c+1)).  Each
    core computes its 1250 output rows locally -> no collectives.
  - agg^T[dim, dst] = sum_j X_j^T @ W_j: X pair-tiles (fp8, SBUF-resident)
    are the STATIONARY matmul operand; the dense weighted-adjacency blocks
    W_j (fp8, host-scattered) stream as the MOVING operand in 2-src-pair
    chunks.  Only 80 distinct stationaries (vs one per matmul): redundant
    compiler-emitted LDWEIGHTS are deleted post-compile, and redundant
    same-semaphore waits are elided so the PE issues back-to-back.
  - agg^T accumulates in PSUM across the whole sweep (10 fp32 [128,256]
    slices = 5 banks).  The sweep is DMA-bound on the 13.1MB W stream.
  - Epilogue exploits LN-affine/projection commutation:
        y = LN(agg) @ W^T = rstd*(agg @ W^T) - (mu*rstd)*(1 @ W^T)
    so no normalized tensor is ever materialized.  Row sums / sumsq come
    from N=1 matmuls against the already-loaded agg^T stationary; the
    per-row scalars for all 10 tiles are computed in one batched DVE pass;
    out = rel + alpha*y folds into two fused elementwise ops (DVE+Pool).
  - rel/out ride in bf16 (host casts) to halve their HBM traffic.
"""

import os
import sys

for _p in ("/opt/trn_rl_repo",):
    if _p not in sys.path:
        sys.path.insert(0, _p)

from contextlib import ExitStack

import numpy as np
import ml_dtypes

import concourse.bacc as bacc
import concourse.mybir as mybir
from concourse.alu_op_type import AluOpType
from concourse.tile import TileContext
from concourse.bass_utils import run_bass_kernel_spmd

F32 = mybir.dt.float32
BF16 = mybir.dt.bfloat16
FP8 = mybir.dt.float8e4
AF = mybir.ActivationFunctionType
DR = mybir.MatmulPerfMode.DoubleRow

N_NODES = 10000
DIM = 256
N_CORES = 8
NODES_PER_CORE = N_NODES // N_CORES  # 1250
TILE = 128
N_TILES = 10            # dst tiles per core (1280 rows incl. 30 pad)
OUT_ROWS = N_TILES * TILE
S_TILES = 80            # 79 src tiles + 1 zero pad -> even count
S_PAIRS = S_TILES // 2  # 40 DoubleRow pairs
N_CHUNKS = 20           # W streamed as 2-src-pair chunks
N_GROUPS = 5            # dst column groups of 256 (moving free dim limit)
ALPHA = 0.1
LN_EPS = 1e-5

_CACHE: dict = {}


def _dedup_ldweights(nc):
    """Delete InstLdweights whose AP matches the previously loaded one.

    The compiler emits one Ldweights per matmul; when consecutive matmuls
    share a stationary operand the PE array already holds it.  Only drops
    duplicates that carry no semaphore waits/updates.
    """
    removed = 0
    for fn in nc.m.functions:
        for blk in fn.blocks:
            insts = blk.instructions
            last_sig = None
            keep = []
            changed = False
            for i in insts:
                tn = type(i).__name__
                if tn == "InstLdweights":
                    si = i.sync_info
                    clean = si is None or (
                        len(si.on_wait) == 0 and len(si.on_update) == 0)
                    sig = (str(i.ins[0]), str(i.perf_mode),
                           str(i.is_transpose))
                    if clean and sig == last_sig:
                        removed += 1
                        changed = True
                        continue
                    last_sig = sig
                elif tn == "InstMatmult":
                    if getattr(i, "is_transpose", False):
                        last_sig = None
                elif tn in ("InstEventSemaphore", "InstDrain", "InstNop",
                            "InstNotify"):
                    pass  # does not clobber the PE weight array
                elif tn == "InstMatmultMx":
                    last_sig = None
                keep.append(i)
            if changed:
                while len(insts):
                    insts.pop()
                for i in keep:
                    insts.append(i)
    return removed


def _elide_redundant_waits(nc):
    """Drop semaphore waits already satisfied by an earlier wait on the
    same engine (same sem, >= value).  Sem counters only increment, so once
    an engine has waited for (s >= v), any later wait (s >= v' <= v) on
    that engine is a no-op.  Tile's own optimize_sems pass is disabled
    upstream; this narrow version only ever compares identical sems."""
    import bass_rust
    dropped = 0
    for fn in nc.m.functions:
        for blk in fn.blocks:
            seen: dict = {}  # (engine, sem_id) -> max value waited
            insts = blk.instructions
            keep = []
            changed = False
            for i in insts:
                si = i.sync_info
                eng = getattr(i, "engine", None)
                if si is not None and len(si.on_wait) > 0 and eng is not None:
                    new_waits = []
                    for w in si.on_wait:
                        if (getattr(w, "wait_mode", None) == "sem-ge-imm"
                                and getattr(w, "wait_reg", None) is None):
                            key = (str(eng), w.id)
                            v = w.wait_value
                            if seen.get(key, -1) >= v:
                                dropped += 1
                                continue
                            seen[key] = max(seen.get(key, -1), v)
                        new_waits.append(w)
                    if len(new_waits) != len(si.on_wait):
                        changed = True
                        if (len(new_waits) == 0 and len(si.on_update) == 0
                                and type(i).__name__ == "InstEventSemaphore"):
                            continue  # whole instruction is now a no-op
                        si.on_wait = new_waits
                keep.append(i)
            if changed:
                while len(insts):
                    insts.pop()
                for i in keep:
                    insts.append(i)
    return dropped


def _build():
    nc = bacc.Bacc("TRN2", target_bir_lowering=False, debug=False,
                   num_devices=N_CORES)

    x_d = nc.dram_tensor("x", [128, S_TILES * DIM], FP8,
                         kind="ExternalInput")
    w_d = nc.dram_tensor("wblk", [N_CHUNKS, 128, 4 * OUT_ROWS], FP8,
                         kind="ExternalInput")
    rel_d = nc.dram_tensor("relslice", [OUT_ROWS, DIM], BF16,
                           kind="ExternalInput")
    pwt_d = nc.dram_tensor("projwT", [128, 2 * DIM], BF16,
                           kind="ExternalInput")
    urep_d = nc.dram_tensor("urep", [128, DIM], BF16, kind="ExternalInput")
    out_d = nc.dram_tensor("out", [OUT_ROWS, DIM], BF16,
                           kind="ExternalOutput")

    with TileContext(nc) as tc, ExitStack() as es:
        const_pool = es.enter_context(tc.tile_pool(name="const", bufs=1))
        wpool = es.enter_context(tc.tile_pool(name="wblk", bufs=3))
        ep_pool = es.enter_context(tc.tile_pool(name="ep", bufs=3))
        ps_agg = es.enter_context(tc.tile_pool(name="ps_agg", bufs=1,
                                               space="PSUM"))
        ps_rs = es.enter_context(tc.tile_pool(name="ps_rs", bufs=1,
                                              space="PSUM"))
        ps_z = es.enter_context(tc.tile_pool(name="ps_z", bufs=2,
                                             space="PSUM"))

        # --- constants / resident inputs ---
        epsb = const_pool.tile([128, 1], F32, tag="epsb")
        nc.vector.memset(epsb[:], LN_EPS)
        ones_col = const_pool.tile([128, 1], BF16, tag="ones")
        nc.vector.memset(ones_col[:], 1.0)
        pwt_sb = const_pool.tile([128, 2, DIM], BF16, tag="pwt")
        nc.scalar.dma_start(pwt_sb[:], pwt_d[:])
        urep_sb = const_pool.tile([128, DIM], BF16, tag="urep")
        nc.scalar.dma_start(urep_sb[:], urep_d[:])
        rel_sb = const_pool.tile([128, N_TILES, DIM], BF16, tag="rel")

        # X fp8, fully resident as the stationary operand; chunked load on
        # the scalar ring so the first pairs land before the sweep starts.
        x_sb = const_pool.tile([128, S_TILES, DIM], FP8, tag="x")
        for i in range(0, S_TILES, 20):
            nc.scalar.dma_start(x_sb[:, i:i + 20, :],
                                x_d[:, i * DIM:(i + 20) * DIM])
        for t in range(N_TILES):
            nc.scalar.dma_start(rel_sb[:, t, :],
                                rel_d[t * 128:(t + 1) * 128, :])

        # --- src sweep: agg^T accumulates in PSUM --------------------------
        # slot (h, g): dims [128h, 128h+128) x dst cols [256g, 256g+256)
        aggT = ps_agg.tile([128, 2 * N_GROUPS, 256], F32, tag="aggT")
        for jj in range(N_CHUNKS):
            w2 = wpool.tile([128, 2, 2, OUT_ROWS], FP8, tag="w")
            nc.sync.dma_start(w2[:], w_d[jj])
            for a in range(2):
                j = 2 * jj + a
                for h in range(2):
                    lhs = x_sb[:, 2 * j:2 * j + 2, 128 * h:128 * (h + 1)]
                    for g in range(N_GROUPS):
                        nc.tensor.matmul(
                            aggT[:, h * N_GROUPS + g, :],
                            lhs,
                            w2[:, a, :, 256 * g:256 * (g + 1)],
                            start=(j == 0), stop=(j == S_PAIRS - 1),
                            perf_mode=DR)

        # --- epilogue ------------------------------------------------------
        # S1a: evacuate agg^T to SBUF bf16 (ACT), per tile slice
        aggT_sb = const_pool.tile([128, N_TILES, 2, 128], BF16, tag="aggTsb")
        for t in range(N_TILES):
            g, half = t // 2, t % 2
            for h in range(2):
                nc.scalar.copy(aggT_sb[:, t, h, :],
                               aggT[:, h * N_GROUPS + g,
                                    128 * half:128 * half + 128])

        # S1b: squares (DVE) + row sums / sumsq via N=1 matmuls against the
        # agg^T stationary.  rs_all[:, t, 0] = sum_f agg^T, [:, t, 1] = sumsq.
        rs_all = ps_rs.tile([128, N_TILES, 2], F32, tag="rs")
        for t in range(N_TILES):
            sq = ep_pool.tile([128, 2, 128], BF16, tag="sq")
            nc.vector.tensor_tensor(sq[:], aggT_sb[:, t, :, :],
                                    aggT_sb[:, t, :, :], AluOpType.mult)
            for k in range(2):
                nc.tensor.matmul(rs_all[:, t, 0:1], aggT_sb[:, t, k, :],
                                 ones_col[:], start=(k == 0), stop=(k == 1))
            for k in range(2):
                nc.tensor.matmul(rs_all[:, t, 1:2], sq[:, k, :],
                                 ones_col[:], start=(k == 0), stop=(k == 1))

        # S2: batched per-row scalars for all tiles in one [128, 10] pass.
        #   mu = rs/256; var = ss/256 - mu^2; rstd = 1/sqrt(var + eps)
        #   s = alpha*rstd;  tcoef = -(alpha/256)*rs*rstd
        rs_sb = ep_pool.tile([128, N_TILES, 2], F32, tag="rs_sb")
        nc.vector.tensor_copy(rs_sb[:], rs_all[:])
        rs_f = rs_sb[:, :, 0]
        ss_f = rs_sb[:, :, 1]
        q = ep_pool.tile([128, N_TILES], F32, tag="q")
        nc.vector.tensor_tensor(q[:], rs_f, rs_f, AluOpType.mult)
        q2 = ep_pool.tile([128, N_TILES], F32, tag="q2")
        nc.vector.tensor_scalar(q2[:], q[:], -1.0 / (DIM * DIM), None,
                                AluOpType.mult)
        v = ep_pool.tile([128, N_TILES], F32, tag="v")
        nc.vector.scalar_tensor_tensor(v[:], ss_f, 1.0 / DIM, q2[:],
                                       AluOpType.mult, AluOpType.add)
        std = ep_pool.tile([128, N_TILES], F32, tag="std")
        nc.scalar.activation(std[:], v[:], AF.Sqrt, bias=epsb[:])
        rstd = ep_pool.tile([128, N_TILES], F32, tag="rstd")
        nc.vector.reciprocal(rstd[:], std[:])
        s_all = ep_pool.tile([128, N_TILES], F32, tag="s_all")
        nc.vector.tensor_scalar(s_all[:], rstd[:], ALPHA, None,
                                AluOpType.mult)
        t0 = ep_pool.tile([128, N_TILES], F32, tag="t0")
        nc.vector.tensor_tensor(t0[:], rs_f, rstd[:], AluOpType.mult)
        t_all = ep_pool.tile([128, N_TILES], F32, tag="t_all")
        nc.vector.tensor_scalar(t_all[:], t0[:], -ALPHA / DIM, None,
                                AluOpType.mult)

        # S3 per tile: z = agg @ W^T (agg^T stationary re-loaded), then
        #   out = rel + s*z + tcoef*u  in two fused elementwise ops.
        for t in range(N_TILES):
            z_ps = ps_z.tile([128, DIM], F32, tag="z")
            for k in range(2):
                nc.tensor.matmul(z_ps[:], aggT_sb[:, t, k, :],
                                 pwt_sb[:, k, :], start=(k == 0),
                                 stop=(k == 1))
            acc = ep_pool.tile([128, DIM], BF16, tag="acc")
            nc.vector.scalar_tensor_tensor(acc[:], z_ps[:],
                                           s_all[:, t:t + 1],
                                           rel_sb[:, t, :],
                                           AluOpType.mult, AluOpType.add)
            out_t = ep_pool.tile([128, DIM], BF16, tag="out")
            nc.vector.scalar_tensor_tensor(out_t[:], urep_sb[:],
                                           t_all[:, t:t + 1], acc[:],
                                           AluOpType.mult, AluOpType.add)
            nc.scalar.dma_start(out_d[t * 128:(t + 1) * 128, :], out_t[:])

    nc.compile()
    if os.environ.get("NO_DEDUP") != "1":
        n1 = 0
        if os.environ.get("NO_LDW_DEDUP") != "1":
            n1 = _dedup_ldweights(nc)
        n2 = 0
        if os.environ.get("NO_ELIDE") != "1":
            n2 = _elide_redundant_waits(nc)
        print(f"[kernel] dedup ldweights: {n1}, elided waits: {n2}")
    if os.environ.get("DUMP_IR"):
        with open("/root/problem/work/ir_dump.txt", "w") as f:
            for fn in nc.m.functions:
                for bi, blk in enumerate(fn.blocks):
                    f.write(f"== block {bi} ==\n")
                    for i in blk.instructions:
                        si = i.sync_info
                        w_ = ([f"{w.ant_name}>={w.wait_value}"
                               for w in si.on_wait] if si else [])
                        u_ = ([f"{u.ant_name}+={u.update_value}"
                               for u in si.on_update] if si else [])
                        f.write(f"{type(i).__name__:24s} eng={i.engine} "
                                f"wait={w_} upd={u_}\n")
    return nc


def _prep(rel_embed, rel_edge_index, rel_edge_weight, proj_w):
    """Host-side sharding/layout: scatter edges into dense per-(src pair)
    weight blocks; lay out rel_embed for SBUF residency."""
    src = np.asarray(rel_edge_index[0], dtype=np.int64)
    dst = np.asarray(rel_edge_index[1], dtype=np.int64)
    w = np.asarray(rel_edge_weight, dtype=np.float32)
    rel = np.asarray(rel_embed, dtype=np.float32)
    pw = np.asarray(proj_w, dtype=np.float32)

    core = dst // NODES_PER_CORE
    drel = dst - core * NODES_PER_CORE
    t = drel // TILE
    d = drel % TILE
    s = src // TILE
    p = src % TILE
    # flat index inside one core's [N_TILES, S_TILES, 128, 128] block array
    flat = ((t * S_TILES + s) * TILE + p) * TILE + d
    blk_sz = N_TILES * S_TILES * TILE * TILE

    w_dev = np.empty((N_CORES, N_CHUNKS, 128, 4 * OUT_ROWS),
                     dtype=ml_dtypes.float8_e4m3)
    for c in range(N_CORES):
        m = core == c
        wc = np.bincount(flat[m], weights=w[m], minlength=blk_sz)
        wc = wc.reshape(N_TILES, S_TILES, TILE, TILE).astype(np.float32)
        # [t, s, p, d] -> [jj, p, (a, q, t*128+d)]
        arr = wc.transpose(1, 2, 0, 3).reshape(S_TILES, 128, OUT_ROWS)
        arr = arr.reshape(N_CHUNKS, 4, 128, OUT_ROWS).transpose(0, 2, 1, 3)
        w_dev[c] = arr.reshape(N_CHUNKS, 128, 4 * OUT_ROWS)

    rel8 = rel.astype(ml_dtypes.float8_e4m3)
    rel8_pad = np.zeros((S_TILES * TILE, DIM), dtype=ml_dtypes.float8_e4m3)
    rel8_pad[:N_NODES] = rel8
    x_dev = np.ascontiguousarray(
        rel8_pad.reshape(S_TILES, TILE, DIM).transpose(1, 0, 2).reshape(
            128, S_TILES * DIM))

    relslice = np.zeros((N_CORES, OUT_ROWS, DIM), dtype=ml_dtypes.bfloat16)
    for c in range(N_CORES):
        relslice[c, :NODES_PER_CORE] = rel[c * NODES_PER_CORE:
                                           (c + 1) * NODES_PER_CORE]
    pwt = pw.T.astype(np.float32)  # [f, o]
    pwt_dev = np.ascontiguousarray(
        pwt.reshape(2, 128, DIM).transpose(1, 0, 2).reshape(
            128, 2 * DIM)).astype(ml_dtypes.bfloat16)
    # u[o] = sum_f W[o, f], replicated across partitions
    u = pw.sum(axis=1).astype(np.float32)
    urep = np.broadcast_to(u, (128, DIM)).astype(ml_dtypes.bfloat16)
    urep = np.ascontiguousarray(urep)

    in_maps = []
    for c in range(N_CORES):
        in_maps.append({
            "x": x_dev,
            "wblk": w_dev[c],
            "relslice": relslice[c],
            "projwT": pwt_dev,
            "urep": urep,
        })
    return in_maps


def kernel(rel_embed, rel_edge_index, rel_edge_weight, proj_w,
           _trace=False):
    in_maps = _prep(rel_embed, rel_edge_index, rel_edge_weight, proj_w)
    nc = _CACHE.get("nc")
    if nc is None:
        nc = _build()
        _CACHE["nc"] = nc
    res = run_bass_kernel_spmd(nc, in_maps, core_ids=list(range(N_CORES)),
                               trace=_trace)
    out = np.concatenate(
        [res.results[c]["out"][:NODES_PER_CORE] for c in range(N_CORES)],
        axis=0)
    if _trace:
        kernel.last_results = res
    return out.astype(np.float32)


# revision 28
# speedup vs baseline: 1.3603x; 1.1140x over previous
"""DualRelGCN message-passing kernel for 8 TRN2 NeuronCores.

Strategy (destination-sharded, collective-free, X-stationary block-dense):
  - LayerNorm is invariant to positive per-row scaling, so LN(agg/denom) ==
    LN(agg): the denominator drops out entirely.
  - Shard edges by dst range: core c owns nodes [1250c, 1250(c+1)).  Each
    core computes its 1250 output rows locally -> no collectives.
  - agg^T[dim, dst] = sum_j X_j^T @ W_j: X pair-tiles (fp8, SBUF-resident)
    are the STATIONARY matmul operand; the dense weighted-adjacency blocks
    W_j (fp8, host-scattered) stream as the MOVING operand in 2-src-pair
    chunks.  Only 80 distinct stationaries (vs one per matmul): redundant
    compiler-emitted LDWEIGHTS are deleted post-compile, and redundant
    same-semaphore waits are elided so the PE issues back-to-back.
  - agg^T accumulates in PSUM across the whole sweep (10 fp32 [128,256]
    slices = 5 banks).  The sweep is DMA-bound on the 13.1MB W stream.
  - Epilogue exploits LN-affine/projection commutation:
        y = LN(agg) @ W^T = rstd*(agg @ W^T) - (mu*rstd)*(1 @ W^T)
    so no normalized tensor is ever materialized.  Row sums / sumsq come
    from N=1 matmuls against the already-loaded agg^T stationary; the
    per-row scalars for all 10 tiles are computed in one batched DVE pass;
    out = rel + alpha*y folds into two fused elementwise ops (DVE+Pool).
  - rel/out ride in bf16 (host casts) to halve their HBM traffic.
"""

import os
import sys

for _p in ("/opt/trn_rl_repo",):
    if _p not in sys.path:
        sys.path.insert(0, _p)

from contextlib import ExitStack

import numpy as np
import ml_dtypes

import concourse.bacc as bacc
import concourse.mybir as mybir
from concourse.alu_op_type import AluOpType
from concourse.tile import TileContext
from concourse.bass_utils import run_bass_kernel_spmd

F32 = mybir.dt.float32
BF16 = mybir.dt.bfloat16
FP8 = mybir.dt.float8e4
AF = mybir.ActivationFunctionType
DR = mybir.MatmulPerfMode.DoubleRow

N_NODES = 10000
DIM = 256
N_CORES = 8
NODES_PER_CORE = N_NODES // N_CORES  # 1250
TILE = 128
N_TILES = 10            # dst tiles per core (1280 rows incl. 30 pad)
OUT_ROWS = N_TILES * TILE
S_TILES = 80            # 79 src tiles + 1 zero pad -> even count
S_PAIRS = S_TILES // 2  # 40 DoubleRow pairs
N_CHUNKS = 10           # W streamed as 4-src-pair chunks
N_GROUPS = 5            # dst column groups of 256 (moving free dim limit)
ALPHA = 0.1
LN_EPS = 1e-5

_CACHE: dict = {}


def _dedup_ldweights(nc):
    """Delete InstLdweights whose AP matches the previously loaded one.

    The compiler emits one Ldweights per matmul; when consecutive matmuls
    share a stationary operand the PE array already holds it.  Only drops
    duplicates that carry no semaphore waits/updates.
    """
    removed = 0
    for fn in nc.m.functions:
        for blk in fn.blocks:
            insts = blk.instructions
            last_sig = None
            keep = []
            changed = False
            for i in insts:
                tn = type(i).__name__
                if tn == "InstLdweights":
                    si = i.sync_info
                    clean = si is None or (
                        len(si.on_wait) == 0 and len(si.on_update) == 0)
                    sig = (str(i.ins[0]), str(i.perf_mode),
                           str(i.is_transpose))
                    if clean and sig == last_sig:
                        removed += 1
                        changed = True
                        continue
                    last_sig = sig
                elif tn == "InstMatmult":
                    if getattr(i, "is_transpose", False):
                        last_sig = None
                elif tn in ("InstEventSemaphore", "InstDrain", "InstNop",
                            "InstNotify"):
                    pass  # does not clobber the PE weight array
                elif tn == "InstMatmultMx":
                    last_sig = None
                keep.append(i)
            if changed:
                while len(insts):
                    insts.pop()
                for i in keep:
                    insts.append(i)
    return removed


def _elide_redundant_waits(nc):
    """Drop semaphore waits already satisfied by an earlier wait on the
    same engine (same sem, >= value).  Sem counters only increment, so once
    an engine has waited for (s >= v), any later wait (s >= v' <= v) on
    that engine is a no-op.  Tile's own optimize_sems pass is disabled
    upstream; this narrow version only ever compares identical sems."""
    import bass_rust
    dropped = 0
    for fn in nc.m.functions:
        for blk in fn.blocks:
            seen: dict = {}  # (engine, sem_id) -> max value waited
            insts = blk.instructions
            keep = []
            changed = False
            for i in insts:
                si = i.sync_info
                eng = getattr(i, "engine", None)
                if si is not None and len(si.on_wait) > 0 and eng is not None:
                    new_waits = []
                    for w in si.on_wait:
                        if (getattr(w, "wait_mode", None) == "sem-ge-imm"
                                and getattr(w, "wait_reg", None) is None):
                            key = (str(eng), w.id)
                            v = w.wait_value
                            if seen.get(key, -1) >= v:
                                dropped += 1
                                continue
                            seen[key] = max(seen.get(key, -1), v)
                        new_waits.append(w)
                    if len(new_waits) != len(si.on_wait):
                        changed = True
                        if (len(new_waits) == 0 and len(si.on_update) == 0
                                and type(i).__name__ == "InstEventSemaphore"):
                            continue  # whole instruction is now a no-op
                        si.on_wait = new_waits
                keep.append(i)
            if changed:
                while len(insts):
                    insts.pop()
                for i in keep:
                    insts.append(i)
    return dropped


def _build():
    nc = bacc.Bacc("TRN2", target_bir_lowering=False, debug=False,
                   num_devices=N_CORES)

    x_d = nc.dram_tensor("x", [128, S_TILES * DIM], FP8,
                         kind="ExternalInput")
    w_d = nc.dram_tensor("wblk", [N_CHUNKS, 128, 8 * OUT_ROWS], FP8,
                         kind="ExternalInput")
    rel_d = nc.dram_tensor("relslice", [OUT_ROWS, DIM], BF16,
                           kind="ExternalInput")
    pwt_d = nc.dram_tensor("projwT", [128, 2 * DIM], BF16,
                           kind="ExternalInput")
    urep_d = nc.dram_tensor("urep", [128, DIM], BF16, kind="ExternalInput")
    out_d = nc.dram_tensor("out", [OUT_ROWS, DIM], BF16,
                           kind="ExternalOutput")

    with TileContext(nc) as tc, ExitStack() as es:
        const_pool = es.enter_context(tc.tile_pool(name="const", bufs=1))
        wpool = es.enter_context(tc.tile_pool(name="wblk", bufs=3))
        ep_pool = es.enter_context(tc.tile_pool(name="ep", bufs=3))
        ps_agg = es.enter_context(tc.tile_pool(name="ps_agg", bufs=1,
                                               space="PSUM"))
        ps_rs = es.enter_context(tc.tile_pool(name="ps_rs", bufs=1,
                                              space="PSUM"))

        # --- constants / resident inputs ---
        # W chunk 0 and X chunk 0 first: the sweep starts as soon as both
        # land.  Epilogue-only tensors (rel/pwt/urep) ride the DVE ring so
        # they never delay the W (sync) / X (scalar) streams.
        x_sb = const_pool.tile([128, S_TILES, DIM], FP8, tag="x")
        w_tiles = []
        for jj in range(min(2, N_CHUNKS)):
            w4 = wpool.tile([128, 4, 2, OUT_ROWS], FP8, tag="w")
            nc.sync.dma_start(w4[:], w_d[jj])
            w_tiles.append(w4)
        nc.scalar.dma_start(x_sb[:, 0:20, :], x_d[:, 0:20 * DIM])

        epsb = const_pool.tile([128, 1], F32, tag="epsb")
        nc.vector.memset(epsb[:], LN_EPS)
        ones_col = const_pool.tile([128, 1], BF16, tag="ones")
        nc.vector.memset(ones_col[:], 1.0)
        for i in range(20, S_TILES, 20):
            nc.scalar.dma_start(x_sb[:, i:i + 20, :],
                                x_d[:, i * DIM:(i + 20) * DIM])
        pwt_sb = const_pool.tile([128, 2, DIM], BF16, tag="pwt")
        nc.scalar.dma_start(pwt_sb[:], pwt_d[:])
        urep_sb = const_pool.tile([128, DIM], BF16, tag="urep")
        nc.scalar.dma_start(urep_sb[:], urep_d[:])
        rel_sb = const_pool.tile([128, N_TILES, DIM], BF16, tag="rel")
        nc.scalar.dma_start(
            rel_sb[:],
            rel_d[:].rearrange("(t p) d -> p t d", t=N_TILES, p=128))

        # --- src sweep: agg^T accumulates in PSUM --------------------------
        # slot (h, g): dims [128h, 128h+128) x dst cols [256g, 256g+256)
        aggT = ps_agg.tile([128, 2 * N_GROUPS, 256], F32, tag="aggT")
        for jj in range(N_CHUNKS):
            if jj < 2:
                w4 = w_tiles[jj]
            else:
                w4 = wpool.tile([128, 4, 2, OUT_ROWS], FP8, tag="w")
                nc.sync.dma_start(w4[:], w_d[jj])
            for a in range(4):
                j = 4 * jj + a
                for h in range(2):
                    lhs = x_sb[:, 2 * j:2 * j + 2, 128 * h:128 * (h + 1)]
                    for g in range(N_GROUPS):
                        nc.tensor.matmul(
                            aggT[:, h * N_GROUPS + g, :],
                            lhs,
                            w4[:, a, :, 256 * g:256 * (g + 1)],
                            start=(j == 0), stop=(j == S_PAIRS - 1),
                            perf_mode=DR)

        # --- epilogue ------------------------------------------------------
        # S1a: evacuate agg^T to SBUF bf16, one [128,256] copy per (h,g)
        # slot, split across ACT and DVE.
        aggT_sb = const_pool.tile([128, 2, N_TILES, 128], BF16, tag="aggTsb")
        for g in range(N_GROUPS):
            nc.scalar.copy(aggT_sb[:, 0, 2 * g:2 * g + 2, :],
                           aggT[:, 0 * N_GROUPS + g, :])
            nc.vector.tensor_copy(aggT_sb[:, 1, 2 * g:2 * g + 2, :],
                                  aggT[:, 1 * N_GROUPS + g, :])

        # S1b per tile: squares (DVE); then against the agg^T stationary
        # (loaded once per (t,k)): z = agg @ W^T into the freed PSUM banks
        # (generation 2 of the aggT tag) and rowsum via an N=1 matmul.
        # rs_all[:, t, 0] = sum_f agg^T, [:, t, 1] = sum_f (agg^T)^2.
        z_all = ps_agg.tile([128, 2 * N_GROUPS, 256], F32, tag="aggT")
        rs_all = ps_rs.tile([128, N_TILES, 2], F32, tag="rs")
        for t in range(N_TILES):
            sq = ep_pool.tile([128, 2, 128], BF16, tag="sq")
            nc.vector.tensor_tensor(sq[:], aggT_sb[:, :, t, :],
                                    aggT_sb[:, :, t, :], AluOpType.mult)
            for k in range(2):
                nc.tensor.matmul(rs_all[:, t, 0:1], aggT_sb[:, k, t, :],
                                 ones_col[:], start=(k == 0), stop=(k == 1))
                nc.tensor.matmul(z_all[:, t, :], aggT_sb[:, k, t, :],
                                 pwt_sb[:, k, :], start=(k == 0),
                                 stop=(k == 1))
            for k in range(2):
                nc.tensor.matmul(rs_all[:, t, 1:2], sq[:, k, :],
                                 ones_col[:], start=(k == 0), stop=(k == 1))

        # S2: batched per-row scalars for all tiles in one [128, 10] pass.
        #   mu = rs/256; var = ss/256 - mu^2; rstd = 1/sqrt(var + eps)
        #   s = alpha*rstd;  tcoef = -(alpha/256)*rs*rstd
        rs_sb = ep_pool.tile([128, N_TILES, 2], F32, tag="rs_sb")
        nc.vector.tensor_copy(rs_sb[:], rs_all[:])
        rs_f = rs_sb[:, :, 0]
        ss_f = rs_sb[:, :, 1]
        q = ep_pool.tile([128, N_TILES], F32, tag="q")
        nc.vector.tensor_tensor(q[:], rs_f, rs_f, AluOpType.mult)
        q2 = ep_pool.tile([128, N_TILES], F32, tag="q2")
        nc.vector.tensor_scalar(q2[:], q[:], -1.0 / (DIM * DIM), None,
                                AluOpType.mult)
        v = ep_pool.tile([128, N_TILES], F32, tag="v")
        nc.vector.scalar_tensor_tensor(v[:], ss_f, 1.0 / DIM, q2[:],
                                       AluOpType.mult, AluOpType.add)
        std = ep_pool.tile([128, N_TILES], F32, tag="std")
        nc.scalar.activation(std[:], v[:], AF.Sqrt, bias=epsb[:])
        rstd = ep_pool.tile([128, N_TILES], F32, tag="rstd")
        nc.vector.reciprocal(rstd[:], std[:])
        s_all = ep_pool.tile([128, N_TILES], F32, tag="s_all")
        nc.vector.tensor_scalar(s_all[:], rstd[:], ALPHA, None,
                                AluOpType.mult)
        t0 = ep_pool.tile([128, N_TILES], F32, tag="t0")
        nc.vector.tensor_tensor(t0[:], rs_f, rstd[:], AluOpType.mult)
        t_all = ep_pool.tile([128, N_TILES], F32, tag="t_all")
        nc.vector.tensor_scalar(t_all[:], t0[:], -ALPHA / DIM, None,
                                AluOpType.mult)

        # S3 per tile: out = rel + s*z + tcoef*u in two fused elementwise
        # ops (DVE), then store.
        for t in range(N_TILES):
            acc = ep_pool.tile([128, DIM], BF16, tag="acc")
            nc.vector.scalar_tensor_tensor(acc[:], z_all[:, t, :],
                                           s_all[:, t:t + 1],
                                           rel_sb[:, t, :],
                                           AluOpType.mult, AluOpType.add)
            out_t = ep_pool.tile([128, DIM], BF16, tag="out")
            nc.vector.scalar_tensor_tensor(out_t[:], urep_sb[:],
                                           t_all[:, t:t + 1], acc[:],
                                           AluOpType.mult, AluOpType.add)
            nc.scalar.dma_start(out_d[t * 128:(t + 1) * 128, :], out_t[:])

    nc.compile()
    if os.environ.get("NO_DEDUP") != "1":
        n1 = 0
        if os.environ.get("NO_LDW_DEDUP") != "1":
            n1 = _dedup_ldweights(nc)
        n2 = 0
        if os.environ.get("DO_ELIDE") == "1":
            n2 = _elide_redundant_waits(nc)
        print(f"[kernel] dedup ldweights: {n1}, elided waits: {n2}")
    if os.environ.get("DUMP_IR"):
        with open("/root/problem/work/ir_dump.txt", "w") as f:
            for fn in nc.m.functions:
                for bi, blk in enumerate(fn.blocks):
                    f.write(f"== block {bi} ==\n")
                    for i in blk.instructions:
                        si = i.sync_info
                        w_ = ([f"{w.ant_name}>={w.wait_value}"
                               for w in si.on_wait] if si else [])
                        u_ = ([f"{u.ant_name}+={u.update_value}"
                               for u in si.on_update] if si else [])
                        f.write(f"{type(i).__name__:24s} eng={i.engine} "
                                f"wait={w_} upd={u_}\n")
    return nc


def _prep(rel_embed, rel_edge_index, rel_edge_weight, proj_w):
    """Host-side sharding/layout: scatter edges into dense per-(src pair)
    weight blocks; lay out rel_embed for SBUF residency."""
    src = np.asarray(rel_edge_index[0], dtype=np.int64)
    dst = np.asarray(rel_edge_index[1], dtype=np.int64)
    w = np.asarray(rel_edge_weight, dtype=np.float32)
    rel = np.asarray(rel_embed, dtype=np.float32)
    pw = np.asarray(proj_w, dtype=np.float32)

    core = dst // NODES_PER_CORE
    drel = dst - core * NODES_PER_CORE
    t = drel // TILE
    d = drel % TILE
    s = src // TILE
    p = src % TILE
    # flat index inside one core's [N_TILES, S_TILES, 128, 128] block array
    flat = ((t * S_TILES + s) * TILE + p) * TILE + d
    blk_sz = N_TILES * S_TILES * TILE * TILE

    w_dev = np.empty((N_CORES, N_CHUNKS, 128, 8 * OUT_ROWS),
                     dtype=ml_dtypes.float8_e4m3)
    for c in range(N_CORES):
        m = core == c
        wc = np.bincount(flat[m], weights=w[m], minlength=blk_sz)
        wc = wc.reshape(N_TILES, S_TILES, TILE, TILE).astype(np.float32)
        # [t, s, p, d] -> [jj, p, (a, q, t*128+d)]
        arr = wc.transpose(1, 2, 0, 3).reshape(S_TILES, 128, OUT_ROWS)
        arr = arr.reshape(N_CHUNKS, 8, 128, OUT_ROWS).transpose(0, 2, 1, 3)
        w_dev[c] = arr.reshape(N_CHUNKS, 128, 8 * OUT_ROWS)

    rel8 = rel.astype(ml_dtypes.float8_e4m3)
    rel8_pad = np.zeros((S_TILES * TILE, DIM), dtype=ml_dtypes.float8_e4m3)
    rel8_pad[:N_NODES] = rel8
    x_dev = np.ascontiguousarray(
        rel8_pad.reshape(S_TILES, TILE, DIM).transpose(1, 0, 2).reshape(
            128, S_TILES * DIM))

    relslice = np.zeros((N_CORES, OUT_ROWS, DIM), dtype=ml_dtypes.bfloat16)
    for c in range(N_CORES):
        relslice[c, :NODES_PER_CORE] = rel[c * NODES_PER_CORE:
                                           (c + 1) * NODES_PER_CORE]
    pwt = pw.T.astype(np.float32)  # [f, o]
    pwt_dev = np.ascontiguousarray(
        pwt.reshape(2, 128, DIM).transpose(1, 0, 2).reshape(
            128, 2 * DIM)).astype(ml_dtypes.bfloat16)
    # u[o] = sum_f W[o, f], replicated across partitions
    u = pw.sum(axis=1).astype(np.float32)
    urep = np.broadcast_to(u, (128, DIM)).astype(ml_dtypes.bfloat16)
    urep = np.ascontiguousarray(urep)

    in_maps = []
    for c in range(N_CORES):
        in_maps.append({
            "x": x_dev,
            "wblk": w_dev[c],
            "relslice": relslice[c],
            "projwT": pwt_dev,
            "urep": urep,
        })
    return in_maps


def kernel(rel_embed, rel_edge_index, rel_edge_weight, proj_w,
           _trace=False):
    in_maps = _prep(rel_embed, rel_edge_index, rel_edge_weight, proj_w)
    nc = _CACHE.get("nc")
    if nc is None:
        nc = _build()
        _CACHE["nc"] = nc
    res = run_bass_kernel_spmd(nc, in_maps, core_ids=list(range(N_CORES)),
                               trace=_trace)
    out = np.concatenate(
        [res.results[c]["out"][:NODES_PER_CORE] for c in range(N_CORES)],
        axis=0)
    if _trace:
        kernel.last_results = res
    return out.astype(np.float32)


# revision 31
# speedup vs baseline: 1.4160x; 1.0410x over previous
"""DualRelGCN message-passing kernel for 8 TRN2 NeuronCores.

Strategy (destination-sharded, collective-free, X-stationary block-dense):
  - LayerNorm is invariant to positive per-row scaling, so LN(agg/denom) ==
    LN(agg): the denominator drops out entirely.
  - Shard edges by dst range: core c owns nodes [1250c, 1250(c+1)).  Each
    core computes its 1250 output rows locally -> no collectives.
  - agg^T[dim, dst] = sum_j X_j^T @ W_j: X pair-tiles (fp8, SBUF-resident)
    are the STATIONARY matmul operand; the dense weighted-adjacency blocks
    W_j (fp8, host-scattered) stream as the MOVING operand in 2-src-pair
    chunks.  Only 80 distinct stationaries (vs one per matmul): redundant
    compiler-emitted LDWEIGHTS are deleted post-compile, and redundant
    same-semaphore waits are elided so the PE issues back-to-back.
  - agg^T accumulates in PSUM across the whole sweep (10 fp32 [128,256]
    slices = 5 banks).  The sweep is DMA-bound on the 13.1MB W stream.
  - Epilogue exploits LN-affine/projection commutation:
        y = LN(agg) @ W^T = rstd*(agg @ W^T) - (mu*rstd)*(1 @ W^T)
    so no normalized tensor is ever materialized.  Row sums / sumsq come
    from N=1 matmuls against the already-loaded agg^T stationary; the
    per-row scalars for all 10 tiles are computed in one batched DVE pass;
    out = rel + alpha*y folds into two fused elementwise ops (DVE+Pool).
  - rel/out ride in bf16 (host casts) to halve their HBM traffic.
"""

import os
import sys

for _p in ("/opt/trn_rl_repo",):
    if _p not in sys.path:
        sys.path.insert(0, _p)

from contextlib import ExitStack

import numpy as np
import ml_dtypes

import concourse.bacc as bacc
import concourse.mybir as mybir
from concourse.alu_op_type import AluOpType
from concourse.tile import TileContext
from concourse.bass_utils import run_bass_kernel_spmd

F32 = mybir.dt.float32
BF16 = mybir.dt.bfloat16
FP8 = mybir.dt.float8e4
AF = mybir.ActivationFunctionType
DR = mybir.MatmulPerfMode.DoubleRow

N_NODES = 10000
DIM = 256
N_CORES = 8
NODES_PER_CORE = N_NODES // N_CORES  # 1250
TILE = 128
N_TILES = 10            # dst tiles per core (1280 rows incl. 30 pad)
OUT_ROWS = N_TILES * TILE
S_TILES = 80            # 79 src tiles + 1 zero pad -> even count
S_PAIRS = S_TILES // 2  # 40 DoubleRow pairs
N_CHUNKS = 10           # W streamed as 4-src-pair chunks
N_GROUPS = 5            # dst column groups of 256 (moving free dim limit)
ALPHA = 0.1
LN_EPS = 1e-5

_CACHE: dict = {}


def _dedup_ldweights(nc):
    """Delete InstLdweights whose AP matches the previously loaded one.

    The compiler emits one Ldweights per matmul; when consecutive matmuls
    share a stationary operand the PE array already holds it.  Only drops
    duplicates that carry no semaphore waits/updates.
    """
    removed = 0
    for fn in nc.m.functions:
        for blk in fn.blocks:
            insts = blk.instructions
            last_sig = None
            keep = []
            changed = False
            for i in insts:
                tn = type(i).__name__
                if tn == "InstLdweights":
                    si = i.sync_info
                    clean = si is None or (
                        len(si.on_wait) == 0 and len(si.on_update) == 0)
                    sig = (str(i.ins[0]), str(i.perf_mode),
                           str(i.is_transpose))
                    if clean and sig == last_sig:
                        removed += 1
                        changed = True
                        continue
                    last_sig = sig
                elif tn == "InstMatmult":
                    if getattr(i, "is_transpose", False):
                        last_sig = None
                elif tn in ("InstEventSemaphore", "InstDrain", "InstNop",
                            "InstNotify"):
                    pass  # does not clobber the PE weight array
                elif tn == "InstMatmultMx":
                    last_sig = None
                keep.append(i)
            if changed:
                while len(insts):
                    insts.pop()
                for i in keep:
                    insts.append(i)
    return removed


def _elide_redundant_waits(nc):
    """Drop semaphore waits already satisfied by an earlier wait on the
    same engine (same sem, >= value).  Sem counters only increment, so once
    an engine has waited for (s >= v), any later wait (s >= v' <= v) on
    that engine is a no-op.  Tile's own optimize_sems pass is disabled
    upstream; this narrow version only ever compares identical sems."""
    import bass_rust
    dropped = 0
    for fn in nc.m.functions:
        for blk in fn.blocks:
            seen: dict = {}  # (engine, sem_id) -> max value waited
            insts = blk.instructions
            keep = []
            changed = False
            for i in insts:
                si = i.sync_info
                eng = getattr(i, "engine", None)
                if si is not None and len(si.on_wait) > 0 and eng is not None:
                    new_waits = []
                    for w in si.on_wait:
                        if (getattr(w, "wait_mode", None) == "sem-ge-imm"
                                and getattr(w, "wait_reg", None) is None):
                            key = (str(eng), w.id)
                            v = w.wait_value
                            if seen.get(key, -1) >= v:
                                dropped += 1
                                continue
                            seen[key] = max(seen.get(key, -1), v)
                        new_waits.append(w)
                    if len(new_waits) != len(si.on_wait):
                        changed = True
                        if (len(new_waits) == 0 and len(si.on_update) == 0
                                and type(i).__name__ == "InstEventSemaphore"):
                            continue  # whole instruction is now a no-op
                        si.on_wait = new_waits
                keep.append(i)
            if changed:
                while len(insts):
                    insts.pop()
                for i in keep:
                    insts.append(i)
    return dropped


def _build():
    nc = bacc.Bacc("TRN2", target_bir_lowering=False, debug=False,
                   num_devices=N_CORES)

    x_d = nc.dram_tensor("x", [128, S_TILES * DIM], FP8,
                         kind="ExternalInput")
    w_d = nc.dram_tensor("wblk", [N_CHUNKS, 128, 8 * OUT_ROWS], FP8,
                         kind="ExternalInput")
    rel_d = nc.dram_tensor("relslice", [OUT_ROWS, DIM], BF16,
                           kind="ExternalInput")
    pwt_d = nc.dram_tensor("projwT", [128, 2 * DIM], BF16,
                           kind="ExternalInput")
    urep_d = nc.dram_tensor("urep", [128, DIM], BF16, kind="ExternalInput")
    out_d = nc.dram_tensor("out", [OUT_ROWS, DIM], BF16,
                           kind="ExternalOutput")

    with TileContext(nc) as tc, ExitStack() as es:
        const_pool = es.enter_context(tc.tile_pool(name="const", bufs=1))
        wpool = es.enter_context(tc.tile_pool(name="wblk", bufs=N_CHUNKS))
        ep_pool = es.enter_context(tc.tile_pool(name="ep", bufs=3))
        ps_agg = es.enter_context(tc.tile_pool(name="ps_agg", bufs=1,
                                               space="PSUM"))
        ps_rs = es.enter_context(tc.tile_pool(name="ps_rs", bufs=1,
                                              space="PSUM"))

        # --- constants / resident inputs ---
        # All W chunks are issued upfront on the sync ring (the whole W
        # stream fits in SBUF), with X chunk 0 wedged in right after W0 so
        # the sweep starts immediately; PE then runs behind the DMA
        # wavefront with no buffer-recycling waits.  Epilogue-only tensors
        # (pwt/urep/rel) ride the scalar ring after the X chunks.
        x_sb = const_pool.tile([128, S_TILES, DIM], FP8, tag="x")
        w_tiles = []
        for jj in range(N_CHUNKS):
            w4 = wpool.tile([128, 4, 2, OUT_ROWS], FP8, tag="w")
            w_tiles.append(w4)
        nc.sync.dma_start(w_tiles[0][:], w_d[0])
        nc.sync.dma_start(x_sb[:, 0:20, :], x_d[:, 0:20 * DIM])
        for jj in range(1, N_CHUNKS):
            nc.sync.dma_start(w_tiles[jj][:], w_d[jj])

        epsb = const_pool.tile([128, 1], F32, tag="epsb")
        nc.vector.memset(epsb[:], LN_EPS)
        ones_col = const_pool.tile([128, 1], BF16, tag="ones")
        nc.vector.memset(ones_col[:], 1.0)
        for i in range(20, S_TILES, 20):
            nc.scalar.dma_start(x_sb[:, i:i + 20, :],
                                x_d[:, i * DIM:(i + 20) * DIM])
        pwt_sb = const_pool.tile([128, 2, DIM], BF16, tag="pwt")
        nc.scalar.dma_start(pwt_sb[:], pwt_d[:])
        urep_sb = const_pool.tile([128, DIM], BF16, tag="urep")
        nc.scalar.dma_start(urep_sb[:], urep_d[:])
        rel_sb = const_pool.tile([128, N_TILES, DIM], BF16, tag="rel")
        nc.scalar.dma_start(
            rel_sb[:],
            rel_d[:].rearrange("(t p) d -> p t d", t=N_TILES, p=128))

        # --- src sweep: agg^T accumulates in PSUM --------------------------
        # slot (h, g): dims [128h, 128h+128) x dst cols [256g, 256g+256)
        aggT = ps_agg.tile([128, 2 * N_GROUPS, 256], F32, tag="aggT")
        for jj in range(N_CHUNKS):
            w4 = w_tiles[jj]
            for a in range(4):
                j = 4 * jj + a
                for h in range(2):
                    lhs = x_sb[:, 2 * j:2 * j + 2, 128 * h:128 * (h + 1)]
                    for g in range(N_GROUPS):
                        nc.tensor.matmul(
                            aggT[:, h * N_GROUPS + g, :],
                            lhs,
                            w4[:, a, :, 256 * g:256 * (g + 1)],
                            start=(j == 0), stop=(j == S_PAIRS - 1),
                            perf_mode=DR)

        # --- epilogue ------------------------------------------------------
        # S1a: evacuate agg^T to SBUF bf16, one [128,256] copy per (h,g)
        # slot, split across ACT and DVE.
        aggT_sb = const_pool.tile([128, 2, N_TILES, 128], BF16, tag="aggTsb")
        for g in range(N_GROUPS):
            nc.scalar.copy(aggT_sb[:, 0, 2 * g:2 * g + 2, :],
                           aggT[:, 0 * N_GROUPS + g, :])
            nc.vector.tensor_copy(aggT_sb[:, 1, 2 * g:2 * g + 2, :],
                                  aggT[:, 1 * N_GROUPS + g, :])

        # S1b per tile: squares (DVE); then against the agg^T stationary
        # (loaded once per (t,k)): z = agg @ W^T into the freed PSUM banks
        # (generation 2 of the aggT tag) and rowsum via an N=1 matmul.
        # rs_all[:, t, 0] = sum_f agg^T, [:, t, 1] = sum_f (agg^T)^2.
        z_all = ps_agg.tile([128, 2 * N_GROUPS, 256], F32, tag="aggT")
        rs_all = ps_rs.tile([128, N_TILES, 2], F32, tag="rs")
        for t in range(N_TILES):
            sq = ep_pool.tile([128, 2, 128], BF16, tag="sq")
            nc.vector.tensor_tensor(sq[:], aggT_sb[:, :, t, :],
                                    aggT_sb[:, :, t, :], AluOpType.mult)
            for k in range(2):
                nc.tensor.matmul(rs_all[:, t, 0:1], aggT_sb[:, k, t, :],
                                 ones_col[:], start=(k == 0), stop=(k == 1))
                nc.tensor.matmul(z_all[:, t, :], aggT_sb[:, k, t, :],
                                 pwt_sb[:, k, :], start=(k == 0),
                                 stop=(k == 1))
            for k in range(2):
                nc.tensor.matmul(rs_all[:, t, 1:2], sq[:, k, :],
                                 ones_col[:], start=(k == 0), stop=(k == 1))

        # S2: batched per-row scalars for all tiles in one [128, 10] pass.
        #   mu = rs/256; var = ss/256 - mu^2; rstd = 1/sqrt(var + eps)
        #   s = alpha*rstd;  tcoef = -(alpha/256)*rs*rstd
        rs_sb = ep_pool.tile([128, N_TILES, 2], F32, tag="rs_sb")
        nc.vector.tensor_copy(rs_sb[:], rs_all[:])
        rs_f = rs_sb[:, :, 0]
        ss_f = rs_sb[:, :, 1]
        q = ep_pool.tile([128, N_TILES], F32, tag="q")
        nc.vector.tensor_tensor(q[:], rs_f, rs_f, AluOpType.mult)
        q2 = ep_pool.tile([128, N_TILES], F32, tag="q2")
        nc.vector.tensor_scalar(q2[:], q[:], -1.0 / (DIM * DIM), None,
                                AluOpType.mult)
        v = ep_pool.tile([128, N_TILES], F32, tag="v")
        nc.vector.scalar_tensor_tensor(v[:], ss_f, 1.0 / DIM, q2[:],
                                       AluOpType.mult, AluOpType.add)
        std = ep_pool.tile([128, N_TILES], F32, tag="std")
        nc.scalar.activation(std[:], v[:], AF.Sqrt, bias=epsb[:])
        rstd = ep_pool.tile([128, N_TILES], F32, tag="rstd")
        nc.vector.reciprocal(rstd[:], std[:])
        s_all = ep_pool.tile([128, N_TILES], F32, tag="s_all")
        nc.vector.tensor_scalar(s_all[:], rstd[:], ALPHA, None,
                                AluOpType.mult)
        t0 = ep_pool.tile([128, N_TILES], F32, tag="t0")
        nc.vector.tensor_tensor(t0[:], rs_f, rstd[:], AluOpType.mult)
        t_all = ep_pool.tile([128, N_TILES], F32, tag="t_all")
        nc.vector.tensor_scalar(t_all[:], t0[:], -ALPHA / DIM, None,
                                AluOpType.mult)

        # S3 per tile: out = rel + s*z + tcoef*u, split ACT / DVE:
        #   m2 = s*z (ACT, per-partition scale), acc = tcoef*u + rel (DVE),
        #   out = acc + m2 (DVE), store on the (now idle) sync ring.
        for t in range(N_TILES):
            m2 = ep_pool.tile([128, DIM], BF16, tag="m2")
            nc.scalar.activation(m2[:], z_all[:, t, :], AF.Copy,
                                 scale=s_all[:, t:t + 1])
            acc = ep_pool.tile([128, DIM], BF16, tag="acc")
            nc.vector.scalar_tensor_tensor(acc[:], urep_sb[:],
                                           t_all[:, t:t + 1],
                                           rel_sb[:, t, :],
                                           AluOpType.mult, AluOpType.add)
            out_t = ep_pool.tile([128, DIM], BF16, tag="out")
            nc.vector.tensor_tensor(out_t[:], acc[:], m2[:], AluOpType.add)
            nc.sync.dma_start(out_d[t * 128:(t + 1) * 128, :], out_t[:])

    nc.compile()
    if os.environ.get("NO_DEDUP") != "1":
        n1 = 0
        if os.environ.get("NO_LDW_DEDUP") != "1":
            n1 = _dedup_ldweights(nc)
        n2 = 0
        if os.environ.get("DO_ELIDE") == "1":
            n2 = _elide_redundant_waits(nc)
        print(f"[kernel] dedup ldweights: {n1}, elided waits: {n2}")
    if os.environ.get("DUMP_IR"):
        with open("/root/problem/work/ir_dump.txt", "w") as f:
            for fn in nc.m.functions:
                for bi, blk in enumerate(fn.blocks):
                    f.write(f"== block {bi} ==\n")
                    for i in blk.instructions:
                        si = i.sync_info
                        w_ = ([f"{w.ant_name}>={w.wait_value}"
                               for w in si.on_wait] if si else [])
                        u_ = ([f"{u.ant_name}+={u.update_value}"
                               for u in si.on_update] if si else [])
                        f.write(f"{type(i).__name__:24s} eng={i.engine} "
                                f"wait={w_} upd={u_}\n")
    return nc


def _prep(rel_embed, rel_edge_index, rel_edge_weight, proj_w):
    """Host-side sharding/layout: scatter edges into dense per-(src pair)
    weight blocks; lay out rel_embed for SBUF residency."""
    src = np.asarray(rel_edge_index[0], dtype=np.int64)
    dst = np.asarray(rel_edge_index[1], dtype=np.int64)
    w = np.asarray(rel_edge_weight, dtype=np.float32)
    rel = np.asarray(rel_embed, dtype=np.float32)
    pw = np.asarray(proj_w, dtype=np.float32)

    core = dst // NODES_PER_CORE
    drel = dst - core * NODES_PER_CORE
    t = drel // TILE
    d = drel % TILE
    s = src // TILE
    p = src % TILE
    # flat index inside one core's [N_TILES, S_TILES, 128, 128] block array
    flat = ((t * S_TILES + s) * TILE + p) * TILE + d
    blk_sz = N_TILES * S_TILES * TILE * TILE

    w_dev = np.empty((N_CORES, N_CHUNKS, 128, 8 * OUT_ROWS),
                     dtype=ml_dtypes.float8_e4m3)
    for c in range(N_CORES):
        m = core == c
        wc = np.bincount(flat[m], weights=w[m], minlength=blk_sz)
        wc = wc.reshape(N_TILES, S_TILES, TILE, TILE).astype(np.float32)
        # [t, s, p, d] -> [jj, p, (a, q, t*128+d)]
        arr = wc.transpose(1, 2, 0, 3).reshape(S_TILES, 128, OUT_ROWS)
        arr = arr.reshape(N_CHUNKS, 8, 128, OUT_ROWS).transpose(0, 2, 1, 3)
        w_dev[c] = arr.reshape(N_CHUNKS, 128, 8 * OUT_ROWS)

    rel8 = rel.astype(ml_dtypes.float8_e4m3)
    rel8_pad = np.zeros((S_TILES * TILE, DIM), dtype=ml_dtypes.float8_e4m3)
    rel8_pad[:N_NODES] = rel8
    x_dev = np.ascontiguousarray(
        rel8_pad.reshape(S_TILES, TILE, DIM).transpose(1, 0, 2).reshape(
            128, S_TILES * DIM))

    relslice = np.zeros((N_CORES, OUT_ROWS, DIM), dtype=ml_dtypes.bfloat16)
    for c in range(N_CORES):
        relslice[c, :NODES_PER_CORE] = rel[c * NODES_PER_CORE:
                                           (c + 1) * NODES_PER_CORE]
    pwt = pw.T.astype(np.float32)  # [f, o]
    pwt_dev = np.ascontiguousarray(
        pwt.reshape(2, 128, DIM).transpose(1, 0, 2).reshape(
            128, 2 * DIM)).astype(ml_dtypes.bfloat16)
    # u[o] = sum_f W[o, f], replicated across partitions
    u = pw.sum(axis=1).astype(np.float32)
    urep = np.broadcast_to(u, (128, DIM)).astype(ml_dtypes.bfloat16)
    urep = np.ascontiguousarray(urep)

    in_maps = []
    for c in range(N_CORES):
        in_maps.append({
            "x": x_dev,
            "wblk": w_dev[c],
            "relslice": relslice[c],
            "projwT": pwt_dev,
            "urep": urep,
        })
    return in_maps


def kernel(rel_embed, rel_edge_index, rel_edge_weight, proj_w,
           _trace=False):
    in_maps = _prep(rel_embed, rel_edge_index, rel_edge_weight, proj_w)
    nc = _CACHE.get("nc")
    if nc is None:
        nc = _build()
        _CACHE["nc"] = nc
    res = run_bass_kernel_spmd(nc, in_maps, core_ids=list(range(N_CORES)),
                               trace=_trace)
    out = np.concatenate(
        [res.results[c]["out"][:NODES_PER_CORE] for c in range(N_CORES)],
        axis=0)
    if _trace:
        kernel.last_results = res
    return out.astype(np.float32)
